# revision 16
# baseline (speedup 1.0000x reference)
"""Trainium2 Bass kernel for multi-head attention (B=2, S=2048, D=1024, H=16).

Sharding: 8 cores = 2 batches x 4 head-groups (Megatron column-parallel QKV /
row-parallel Wo). Each core computes a partial (S, D) output for its batch;
the host sums the 4 head-group partials per batch and adds bo.

Per-core pipeline (all on one NeuronCore):
  - QKV projections as fp32r matmuls (full rate, FP22 mantissa).
  - Scores computed transposed (t on partitions, s on free dim):
      scoresT[t, s] = sum_d khT[d, t] * qhT[d, s]
  - exp on ScalarE with the 1/sqrt(dh) scale folded in; no max-subtraction
    (scores ~ N(0,1) after scaling, exp can't overflow).
  - mask applied multiplicatively after exp (mask is 0/1) on VectorE in bf16.
  - PV matmul with a ones-augmented V (65th column) so the softmax
    denominator accumulates in the same PSUM tile.
  - normalize with DVE reciprocal + partition-broadcast DMA, then the Wo
    projection on-device.
"""

import os
import sys

for p in ("/opt/trn_rl_repo", "/root/.axon_site", "/root/.axon_site/_ro/trn_rl_repo"):
    if os.path.isdir(p) and p not in sys.path:
        sys.path.append(p)

import numpy as np
import ml_dtypes

import concourse.bass as bass
import concourse.mybir as mybir
import concourse.tile as tile
from concourse import bacc
from concourse.bass import ts, ds
from concourse.bass_utils import run_bass_kernel_spmd

F32 = mybir.dt.float32
F32R = mybir.dt.float32r
BF16 = mybir.dt.bfloat16

D = 1024          # model dim
DH = 64           # head dim
HPC = 4           # heads per core
DSH = HPC * DH    # sharded projection dim (256)
P = 128

AF = mybir.ActivationFunctionType


def build_nc(S=2048, n_cores=8):
    """Build the single-core SPMD Bass program."""
    SC = 512                  # s-chunk (matmul free dim)
    N_SC = S // SC            # s-chunks
    TT = S // P               # t tiles of 128
    G = 4                     # t-tiles per exp group (4 PSUM banks)
    N_G = TT // G
    KD = D // P               # contraction tiles for projections

    nc = bacc.Bacc(
        trn_type="TRN2",
        target_bir_lowering=False,
        debug=False,
        enable_asserts=False,
        num_devices=n_cores,
    )

    # DRAM I/O (per-core shard shapes)
    qT = nc.dram_tensor("qT", [D, S], BF16, kind="ExternalInput").ap()
    kT = nc.dram_tensor("kT", [D, S], BF16, kind="ExternalInput").ap()
    vT = nc.dram_tensor("vT", [D, S], BF16, kind="ExternalInput").ap()
    maskT = nc.dram_tensor("maskT", [S, S], BF16, kind="ExternalInput").ap()
    WqT = nc.dram_tensor("WqT", [D, DSH], BF16, kind="ExternalInput").ap()
    WkT = nc.dram_tensor("WkT", [D, DSH], BF16, kind="ExternalInput").ap()
    WvT = nc.dram_tensor("WvT", [D, DSH], BF16, kind="ExternalInput").ap()
    WoT = nc.dram_tensor("WoT", [DSH, D], BF16, kind="ExternalInput").ap()
    bq = nc.dram_tensor("bq", [DSH], F32, kind="ExternalInput").ap()
    bk = nc.dram_tensor("bk", [DSH], F32, kind="ExternalInput").ap()
    bv = nc.dram_tensor("bv", [DSH], F32, kind="ExternalInput").ap()
    out = nc.dram_tensor("out", [S, D], F32, kind="ExternalOutput").ap()

    with tile.TileContext(nc) as tc:
        _emit(tc, nc, S, SC, N_SC, TT, G, N_G, KD,
              qT, kT, vT, maskT, WqT, WkT, WvT, WoT, bq, bk, bv, out)
    nc.compile()
    return nc


def _emit(tc, nc, S, SC, N_SC, TT, G, N_G, KD,
          qT, kT, vT, maskT, WqT, WkT, WvT, WoT, bq, bk, bv, out):
    from contextlib import ExitStack

    from concourse import library_config

    ctx = ExitStack()
    with ctx:
        consts = ctx.enter_context(tc.tile_pool(name="consts", bufs=1))
        dram_scratch = ctx.enter_context(
            tc.tile_pool(name="dscratch", bufs=2, space="DRAM"))
        xstream = ctx.enter_context(tc.tile_pool(name="xstream", bufs=3))
        mask_pool = ctx.enter_context(tc.tile_pool(name="mask", bufs=2))
        attn_pool = ctx.enter_context(tc.tile_pool(name="attn", bufs=3))
        small = ctx.enter_context(tc.tile_pool(name="small", bufs=2))
        out_pool = ctx.enter_context(tc.tile_pool(name="outp", bufs=2))

        # ---- resident tensors ----
        wq_sb = consts.tile([P, KD, DSH], BF16)
        nc.sync.dma_start(wq_sb, WqT.rearrange("(kt p) m -> p kt m", p=P))
        wk_sb = consts.tile([P, KD, DSH], BF16)
        nc.sync.dma_start(wk_sb, WkT.rearrange("(kt p) m -> p kt m", p=P))
        wv_sb = consts.tile([P, KD, DSH], BF16)
        nc.sync.dma_start(wv_sb, WvT.rearrange("(kt p) m -> p kt m", p=P))
        # WoT [DSH, D] -> [64, head, D]
        wo_sb = consts.tile([DH, HPC, D], BF16)
        nc.sync.dma_start(wo_sb, WoT.rearrange("(h p) n -> p h n", p=DH))

        bq_sb = consts.tile([P, DSH // P], F32)
        nc.sync.dma_start(bq_sb, bq.rearrange("(c p) -> p c", p=P))
        bk_sb = consts.tile([P, DSH // P], F32)
        nc.sync.dma_start(bk_sb, bk.rearrange("(c p) -> p c", p=P))
        # bv broadcast along partitions (added along the free dim of vh)
        bv_sb = consts.tile([P, DSH], F32)
        nc.sync.dma_start(bv_sb, bv[None, :].to_broadcast([P, DSH]))

        qhT = consts.tile([P, DSH // P, S], BF16)   # [dh%128, dh//128, s]
        khT = consts.tile([P, DSH // P, S], BF16)
        # V with ones column per head: [t-part, t-tile, h*65+d]
        vh_aug = consts.tile([P, TT, HPC * (DH + 1)], BF16)
        for h in range(HPC):
            nc.vector.memset(vh_aug[:, :, h * 65 + 64: h * 65 + 65], 1.0)
        # normalized attention outputs, heads on free dim: [64, h, s]
        ao_all = consts.tile([DH, HPC, S], BF16)

        qT_t = qT.rearrange("(kt p) s -> p kt s", p=P)
        kT_t = kT.rearrange("(kt p) s -> p kt s", p=P)
        vT_t = vT.rearrange("(kt p) s -> p kt s", p=P)

        # ---- phase A: projections ----
        with tc.tile_pool(name="psA", bufs=3, space="PSUM") as psA:
            for (x_t, w_sb, b_sb, xhT) in ((qT_t, wq_sb, bq_sb, qhT),
                                           (kT_t, wk_sb, bk_sb, khT)):
                for sc in range(N_SC):
                    pss = [psA.tile([P, SC], F32, tag="psA", name=f"psA{c}")
                           for c in range(2)]
                    for kt in range(KD):
                        rhs = xstream.tile([P, SC], BF16, tag="xT")
                        nc.sync.dma_start(rhs, x_t[:, kt, ts(sc, SC)])
                        for c in range(2):
                            nc.tensor.matmul(
                                pss[c], w_sb[:, kt, ts(c, P)], rhs,
                                start=(kt == 0), stop=(kt == KD - 1))
                    for c in range(2):
                        nc.scalar.activation(
                            xhT[:, c, ts(sc, SC)], pss[c], AF.Identity,
                            bias=b_sb[:, c:c + 1], scale=1.0)

            # V projection: vh[t, dh] orientation
            for tt in range(TT):
                psv = psA.tile([P, DSH], F32, tag="psA")
                for kt in range(KD):
                    lhsT = xstream.tile([P, P], BF16, tag="vT")
                    nc.sync.dma_start(lhsT, vT_t[:, kt, ts(tt, P)])
                    nc.tensor.matmul(psv, lhsT, wv_sb[:, kt, :],
                                     start=(kt == 0), stop=(kt == KD - 1))
                for h in range(HPC):
                    nc.vector.tensor_add(
                        vh_aug[:, tt, h * 65: h * 65 + 64],
                        psv[:, ts(h, DH)], bv_sb[:, ts(h, DH)])

        # ---- phases B+C ----
        scores_pool = ctx.enter_context(
            tc.tile_pool(name="psScores", bufs=1, space="PSUM"))
        ao_pool = ctx.enter_context(
            tc.tile_pool(name="psAo", bufs=2, space="PSUM"))
        psC = ctx.enter_context(
            tc.tile_pool(name="psC", bufs=2, space="PSUM"))

        maskT_t = maskT.rearrange("(tt p) s -> p tt s", p=P)
        for sc in range(N_SC):
            mask_sb = mask_pool.tile([P, TT, SC], BF16)
            nc.sync.dma_start(mask_sb, maskT_t[:, :, ts(sc, SC)])

            ao_un = small.tile([DH, HPC, SC], F32, tag="ao_un")
            den4 = small.tile([1, HPC, SC], F32, tag="den4")
            for h in range(HPC):
                hp = (h % 2) * DH     # partition base within chunk
                c = h // 2
                ao_ps = ao_pool.tile([DH + 1, SC], F32)
                for g in range(N_G):
                    sc_ps = scores_pool.tile([P, G, SC], F32)
                    for i in range(G):
                        tt = g * G + i
                        nc.tensor.matmul(
                            sc_ps[:, i, :],
                            khT[hp:hp + DH, c, ts(tt, P)],
                            qhT[hp:hp + DH, c, ts(sc, SC)],
                            start=True, stop=True)
                    attn = attn_pool.tile([P, G, SC], BF16)
                    nc.scalar.activation(attn, sc_ps, AF.Exp, scale=0.125)
                    nc.vector.tensor_mul(attn, attn,
                                         mask_sb[:, ts(g, G), :])
                    for i in range(G):
                        tt = g * G + i
                        nc.tensor.matmul(
                            ao_ps, vh_aug[:, tt, h * 65: h * 65 + 65],
                            attn[:, i, :],
                            start=(tt == 0), stop=(tt == TT - 1))
                # evacuate PSUM right away (frees the ao bank quickly)
                nc.vector.tensor_copy(ao_un[:, h, :], ao_ps[0:DH, :])
                nc.vector.tensor_copy(den4[:, h, :], ao_ps[DH:DH + 1, :])

            # batched normalization for all 4 heads: 1/x = exp(-ln(x)) on
            # ScalarE (one Ln->Exp table swap per s-chunk instead of per
            # head; DVE reciprocal would stall the mask-mult stream and the
            # custom-DVE approx is broken on HW). Broadcast via DRAM bounce
            # on the idle GpSimd DMA queue so the Sync DMA FIFO never blocks.
            lnd = small.tile([1, HPC, SC], F32, tag="lnd")
            nc.scalar.activation(lnd, den4, AF.Ln)
            recip = small.tile([1, HPC, SC], F32, tag="recip")
            nc.scalar.activation(recip, lnd, AF.Exp, scale=-1.0)
            rd = dram_scratch.tile([HPC, SC], F32)
            nc.gpsimd.dma_start(rd, recip)
            for h in range(HPC):
                rb = small.tile([DH, SC], F32, tag="recipb")
                nc.gpsimd.dma_start(rb, rd[h:h + 1].to_broadcast([DH, SC]))
                nc.vector.tensor_mul(ao_all[:, h, ts(sc, SC)],
                                     ao_un[:, h, :], rb)

            # output projection for this s-chunk
            for st in range(SC // P):
                s0 = sc * SC + st * P
                for do in range(D // SC):
                    ops = psC.tile([P, SC], F32)
                    for h in range(HPC):
                        nc.tensor.matmul(
                            ops, ao_all[:, h, ds(s0, P)],
                            wo_sb[:, h, ts(do, SC)],
                            start=(h == 0), stop=(h == HPC - 1))
                    ot = out_pool.tile([P, SC], F32)
                    nc.vector.tensor_copy(ot, ops)
                    nc.sync.dma_start(out[ds(s0, P), ts(do, SC)], ot)


_NC_CACHE = {}

# test.py can set this to request a profiled run; exec time lands in
# LAST_EXEC_NS after the call.
TRACE = False
LAST_EXEC_NS = None
LAST_RESULTS = None


def make_in_maps(q, k, v, mask, Wq, bq, Wk, bk, Wv, bv, Wo, bo):
    B = q.shape[0]
    in_maps = []
    for b in range(B):
        qTb = np.ascontiguousarray(np.asarray(q[b], np.float32).T).astype(ml_dtypes.bfloat16)
        kTb = np.ascontiguousarray(np.asarray(k[b], np.float32).T).astype(ml_dtypes.bfloat16)
        vTb = np.ascontiguousarray(np.asarray(v[b], np.float32).T).astype(ml_dtypes.bfloat16)
        mTb = np.ascontiguousarray(
            np.asarray(mask[b, 0], np.float32).T).astype(ml_dtypes.bfloat16)
        for hg in range(4):
            sl = slice(hg * DSH, (hg + 1) * DSH)
            in_maps.append({
                "qT": qTb, "kT": kTb, "vT": vTb, "maskT": mTb,
                "WqT": np.ascontiguousarray(np.asarray(Wq, np.float32)[sl].T).astype(ml_dtypes.bfloat16),
                "WkT": np.ascontiguousarray(np.asarray(Wk, np.float32)[sl].T).astype(ml_dtypes.bfloat16),
                "WvT": np.ascontiguousarray(np.asarray(Wv, np.float32)[sl].T).astype(ml_dtypes.bfloat16),
                "WoT": np.ascontiguousarray(np.asarray(Wo, np.float32)[:, sl].T).astype(ml_dtypes.bfloat16),
                "bq": np.asarray(bq, np.float32)[sl].copy(),
                "bk": np.asarray(bk, np.float32)[sl].copy(),
                "bv": np.asarray(bv, np.float32)[sl].copy(),
            })
    return in_maps


def kernel(q, k, v, mask, Wq, bq, Wk, bk, Wv, bv, Wo, bo):
    global LAST_EXEC_NS, LAST_RESULTS
    q = np.asarray(q)
    B, S, _ = q.shape
    key = S
    if key not in _NC_CACHE:
        _NC_CACHE[key] = build_nc(S=S)
    nc = _NC_CACHE[key]

    in_maps = make_in_maps(q, k, v, mask, Wq, bq, Wk, bk, Wv, bv, Wo, bo)
    res = run_bass_kernel_spmd(
        nc, in_maps, core_ids=list(range(len(in_maps))), trace=TRACE)
    LAST_EXEC_NS = res.exec_time_ns
    LAST_RESULTS = res
    outs = [r["out"] for r in res.results]
    bo_np = np.asarray(bo, np.float32)
    full = np.zeros((B, S, D), np.float32)
    for b in range(B):
        acc = outs[b * 4].astype(np.float32).copy()
        for hg in range(1, 4):
            acc += outs[b * 4 + hg]
        full[b] = acc + bo_np
    return full


# revision 18
# speedup vs baseline: 1.5675x; 1.5675x over previous
"""Trainium2 Bass kernel for multi-head attention (B=2, S=2048, D=1024, H=16).

Sharding: 8 cores = 2 batches x 4 head-groups (Megatron column-parallel QKV /
row-parallel Wo). Each core computes a partial (S, D) output for its batch;
the host sums the 4 head-group partials per batch and adds bo.

Per-core pipeline (all on one NeuronCore):
  - QKV projections as fp32r matmuls (full rate, FP22 mantissa).
  - Scores computed transposed (t on partitions, s on free dim):
      scoresT[t, s] = sum_d khT[d, t] * qhT[d, s]
  - exp on ScalarE with the 1/sqrt(dh) scale folded in; no max-subtraction
    (scores ~ N(0,1) after scaling, exp can't overflow).
  - mask applied multiplicatively after exp (mask is 0/1) on VectorE in bf16.
  - PV matmul with a ones-augmented V (65th column) so the softmax
    denominator accumulates in the same PSUM tile.
  - normalize with DVE reciprocal + partition-broadcast DMA, then the Wo
    projection on-device.
"""

import os
import sys

for p in ("/opt/trn_rl_repo", "/root/.axon_site", "/root/.axon_site/_ro/trn_rl_repo"):
    if os.path.isdir(p) and p not in sys.path:
        sys.path.append(p)

import numpy as np
import ml_dtypes

import concourse.bass as bass
import concourse.mybir as mybir
import concourse.tile as tile
from concourse import bacc
from concourse.bass import ts, ds
from concourse.bass_utils import run_bass_kernel_spmd

F32 = mybir.dt.float32
F32R = mybir.dt.float32r
BF16 = mybir.dt.bfloat16

D = 1024          # model dim
DH = 64           # head dim
HPC = 4           # heads per core
DSH = HPC * DH    # sharded projection dim (256)
P = 128

AF = mybir.ActivationFunctionType


def build_nc(S=2048, n_cores=8):
    """Build the single-core SPMD Bass program."""
    SC = 512                  # s-chunk (matmul free dim)
    N_SC = S // SC            # s-chunks
    TT = S // P               # t tiles of 128
    G = 2                     # t-tiles per exp group (2 PSUM banks)
    N_G = TT // G
    KD = D // P               # contraction tiles for projections

    nc = bacc.Bacc(
        trn_type="TRN2",
        target_bir_lowering=False,
        debug=False,
        enable_asserts=False,
        num_devices=n_cores,
    )

    # DRAM I/O (per-core shard shapes)
    qT = nc.dram_tensor("qT", [D, S], BF16, kind="ExternalInput").ap()
    kT = nc.dram_tensor("kT", [D, S], BF16, kind="ExternalInput").ap()
    vT = nc.dram_tensor("vT", [D, S], BF16, kind="ExternalInput").ap()
    maskT = nc.dram_tensor("maskT", [S, S], BF16, kind="ExternalInput").ap()
    WqT = nc.dram_tensor("WqT", [D, DSH], BF16, kind="ExternalInput").ap()
    WkT = nc.dram_tensor("WkT", [D, DSH], BF16, kind="ExternalInput").ap()
    WvT = nc.dram_tensor("WvT", [D, DSH], BF16, kind="ExternalInput").ap()
    WoT = nc.dram_tensor("WoT", [DSH, D], BF16, kind="ExternalInput").ap()
    bq = nc.dram_tensor("bq", [DSH], F32, kind="ExternalInput").ap()
    bk = nc.dram_tensor("bk", [DSH], F32, kind="ExternalInput").ap()
    bv = nc.dram_tensor("bv", [DSH], F32, kind="ExternalInput").ap()
    out = nc.dram_tensor("out", [S, D], F32, kind="ExternalOutput").ap()

    with tile.TileContext(nc) as tc:
        _emit(tc, nc, S, SC, N_SC, TT, G, N_G, KD,
              qT, kT, vT, maskT, WqT, WkT, WvT, WoT, bq, bk, bv, out)
    nc.compile()
    return nc


def _emit(tc, nc, S, SC, N_SC, TT, G, N_G, KD,
          qT, kT, vT, maskT, WqT, WkT, WvT, WoT, bq, bk, bv, out):
    from contextlib import ExitStack

    from concourse import library_config

    ctx = ExitStack()
    with ctx:
        consts = ctx.enter_context(tc.tile_pool(name="consts", bufs=1))
        dram_scratch = ctx.enter_context(
            tc.tile_pool(name="dscratch", bufs=2, space="DRAM"))
        xstream = ctx.enter_context(tc.tile_pool(name="xstream", bufs=2))
        mask_pool = ctx.enter_context(tc.tile_pool(name="mask", bufs=2))
        attn_pool = ctx.enter_context(tc.tile_pool(name="attn", bufs=3))
        small = ctx.enter_context(tc.tile_pool(name="small", bufs=2))
        norm1 = ctx.enter_context(tc.tile_pool(name="norm1", bufs=1))
        out_pool = ctx.enter_context(tc.tile_pool(name="outp", bufs=2))

        # ---- resident tensors ----
        wq_sb = consts.tile([P, KD, DSH], BF16)
        nc.sync.dma_start(wq_sb, WqT.rearrange("(kt p) m -> p kt m", p=P))
        wk_sb = consts.tile([P, KD, DSH], BF16)
        nc.sync.dma_start(wk_sb, WkT.rearrange("(kt p) m -> p kt m", p=P))
        wv_sb = consts.tile([P, KD, DSH], BF16)
        nc.sync.dma_start(wv_sb, WvT.rearrange("(kt p) m -> p kt m", p=P))
        # WoT [DSH, D] -> [64, head, D]
        wo_sb = consts.tile([DH, HPC, D], BF16)
        nc.sync.dma_start(wo_sb, WoT.rearrange("(h p) n -> p h n", p=DH))

        bq_sb = consts.tile([P, DSH // P], F32)
        nc.sync.dma_start(bq_sb, bq.rearrange("(c p) -> p c", p=P))
        bk_sb = consts.tile([P, DSH // P], F32)
        nc.sync.dma_start(bk_sb, bk.rearrange("(c p) -> p c", p=P))
        # bv broadcast along partitions (added along the free dim of vh)
        bv_sb = consts.tile([P, DSH], F32)
        nc.sync.dma_start(bv_sb, bv[None, :].to_broadcast([P, DSH]))

        qhT = consts.tile([P, DSH // P, S], BF16)   # [dh%128, dh//128, s]
        khT = consts.tile([P, DSH // P, S], BF16)
        # V with ones column per head: [t-part, t-tile, h*65+d]
        vh_aug = consts.tile([P, TT, HPC * (DH + 1)], BF16)
        for h in range(HPC):
            nc.vector.memset(vh_aug[:, :, h * 65 + 64: h * 65 + 65], 1.0)
        # normalized attention outputs, heads on free dim: [64, h, s]
        ao_all = consts.tile([DH, HPC, S], BF16)

        qT_t = qT.rearrange("(kt p) s -> p kt s", p=P)
        kT_t = kT.rearrange("(kt p) s -> p kt s", p=P)
        vT_t = vT.rearrange("(kt p) s -> p kt s", p=P)

        # ---- phase A: projections ----
        with tc.tile_pool(name="psA", bufs=3, space="PSUM") as psA:
            for (x_t, w_sb, b_sb, xhT) in ((qT_t, wq_sb, bq_sb, qhT),
                                           (kT_t, wk_sb, bk_sb, khT)):
                for sc in range(N_SC):
                    stage = xstream.tile([P, KD, SC], BF16, tag="xstage")
                    nc.sync.dma_start(stage, x_t[:, :, ts(sc, SC)])
                    pss = [psA.tile([P, SC], F32, tag="psA", name=f"psA{c}")
                           for c in range(2)]
                    for kt in range(KD):
                        for c in range(2):
                            nc.tensor.matmul(
                                pss[c], w_sb[:, kt, ts(c, P)], stage[:, kt, :],
                                start=(kt == 0), stop=(kt == KD - 1))
                    for c in range(2):
                        nc.scalar.activation(
                            xhT[:, c, ts(sc, SC)], pss[c], AF.Identity,
                            bias=b_sb[:, c:c + 1], scale=1.0)

            # V projection: vh[t, dh] orientation
            for tg in range(TT // 4):
                vstage = xstream.tile([P, KD, 4 * P], BF16, tag="vstage")
                nc.sync.dma_start(vstage, vT_t[:, :, ts(tg, 4 * P)])
                for tt4 in range(4):
                    tt = tg * 4 + tt4
                    psv = psA.tile([P, DSH], F32, tag="psA")
                    for kt in range(KD):
                        nc.tensor.matmul(psv, vstage[:, kt, ts(tt4, P)],
                                         wv_sb[:, kt, :],
                                         start=(kt == 0), stop=(kt == KD - 1))
                    for h in range(HPC):
                        nc.vector.tensor_add(
                            vh_aug[:, tt, h * 65: h * 65 + 64],
                            psv[:, ts(h, DH)], bv_sb[:, ts(h, DH)])

        # ---- phases B+C ----
        scores_pool = ctx.enter_context(
            tc.tile_pool(name="psScores", bufs=2, space="PSUM"))
        ao_pool = ctx.enter_context(
            tc.tile_pool(name="psAo", bufs=2, space="PSUM"))
        psC = ctx.enter_context(
            tc.tile_pool(name="psC", bufs=2, space="PSUM"))

        maskT_t = maskT.rearrange("(tt p) s -> p tt s", p=P)
        for sc in range(N_SC):
            mask_sb = mask_pool.tile([P, TT, SC], BF16)
            nc.sync.dma_start(mask_sb, maskT_t[:, :, ts(sc, SC)])

            ao_un = small.tile([DH, HPC, SC], F32, tag="ao_un")
            den4 = norm1.tile([1, HPC, SC], F32, tag="den4")
            for h in range(HPC):
                hp = (h % 2) * DH     # partition base within chunk
                c = h // 2
                ao_ps = ao_pool.tile([DH + 1, SC], F32)
                for g in range(N_G):
                    sc_ps = scores_pool.tile([P, G, SC], F32)
                    for i in range(G):
                        tt = g * G + i
                        nc.tensor.matmul(
                            sc_ps[:, i, :],
                            khT[hp:hp + DH, c, ts(tt, P)],
                            qhT[hp:hp + DH, c, ts(sc, SC)],
                            start=True, stop=True)
                    attn = attn_pool.tile([P, G, SC], BF16)
                    nc.scalar.activation(attn, sc_ps, AF.Exp, scale=0.125)
                    nc.vector.tensor_mul(attn, attn,
                                         mask_sb[:, ts(g, G), :])
                    for i in range(G):
                        tt = g * G + i
                        nc.tensor.matmul(
                            ao_ps, vh_aug[:, tt, h * 65: h * 65 + 65],
                            attn[:, i, :],
                            start=(tt == 0), stop=(tt == TT - 1))
                # evacuate PSUM right away (frees the ao bank quickly)
                nc.vector.tensor_copy(ao_un[:, h, :], ao_ps[0:DH, :])
                nc.vector.tensor_copy(den4[:, h, :], ao_ps[DH:DH + 1, :])

            # batched normalization for all 4 heads: 1/x = exp(-ln(x)) on
            # ScalarE (one Ln->Exp table swap per s-chunk instead of per
            # head; DVE reciprocal would stall the mask-mult stream and the
            # custom-DVE approx is broken on HW). Broadcast via DRAM bounce
            # on the idle GpSimd DMA queue so the Sync DMA FIFO never blocks.
            lnd = norm1.tile([1, HPC, SC], F32, tag="lnd")
            nc.scalar.activation(lnd, den4, AF.Ln)
            recip = norm1.tile([1, HPC, SC], F32, tag="recip")
            nc.scalar.activation(recip, lnd, AF.Exp, scale=-1.0)
            rd = dram_scratch.tile([HPC, SC], F32)
            nc.gpsimd.dma_start(rd, recip)
            for h in range(HPC):
                rb = small.tile([DH, SC], F32, tag="recipb")
                nc.gpsimd.dma_start(rb, rd[h:h + 1].to_broadcast([DH, SC]))
                nc.vector.tensor_mul(ao_all[:, h, ts(sc, SC)],
                                     ao_un[:, h, :], rb)

            # output projection for this s-chunk
            for st in range(SC // P):
                s0 = sc * SC + st * P
                for do in range(D // SC):
                    ops = psC.tile([P, SC], F32)
                    for h in range(HPC):
                        nc.tensor.matmul(
                            ops, ao_all[:, h, ds(s0, P)],
                            wo_sb[:, h, ts(do, SC)],
                            start=(h == 0), stop=(h == HPC - 1))
                    ot = out_pool.tile([P, SC], F32)
                    nc.vector.tensor_copy(ot, ops)
                    nc.sync.dma_start(out[ds(s0, P), ts(do, SC)], ot)


_NC_CACHE = {}

# test.py can set this to request a profiled run; exec time lands in
# LAST_EXEC_NS after the call.
TRACE = False
LAST_EXEC_NS = None
LAST_RESULTS = None


def make_in_maps(q, k, v, mask, Wq, bq, Wk, bk, Wv, bv, Wo, bo):
    B = q.shape[0]
    in_maps = []
    for b in range(B):
        qTb = np.ascontiguousarray(np.asarray(q[b], np.float32).T).astype(ml_dtypes.bfloat16)
        kTb = np.ascontiguousarray(np.asarray(k[b], np.float32).T).astype(ml_dtypes.bfloat16)
        vTb = np.ascontiguousarray(np.asarray(v[b], np.float32).T).astype(ml_dtypes.bfloat16)
        mTb = np.ascontiguousarray(
            np.asarray(mask[b, 0], np.float32).T).astype(ml_dtypes.bfloat16)
        for hg in range(4):
            sl = slice(hg * DSH, (hg + 1) * DSH)
            in_maps.append({
                "qT": qTb, "kT": kTb, "vT": vTb, "maskT": mTb,
                "WqT": np.ascontiguousarray(np.asarray(Wq, np.float32)[sl].T).astype(ml_dtypes.bfloat16),
                "WkT": np.ascontiguousarray(np.asarray(Wk, np.float32)[sl].T).astype(ml_dtypes.bfloat16),
                "WvT": np.ascontiguousarray(np.asarray(Wv, np.float32)[sl].T).astype(ml_dtypes.bfloat16),
                "WoT": np.ascontiguousarray(np.asarray(Wo, np.float32)[:, sl].T).astype(ml_dtypes.bfloat16),
                "bq": np.asarray(bq, np.float32)[sl].copy(),
                "bk": np.asarray(bk, np.float32)[sl].copy(),
                "bv": np.asarray(bv, np.float32)[sl].copy(),
            })
    return in_maps


def kernel(q, k, v, mask, Wq, bq, Wk, bk, Wv, bv, Wo, bo):
    global LAST_EXEC_NS, LAST_RESULTS
    q = np.asarray(q)
    B, S, _ = q.shape
    key = S
    if key not in _NC_CACHE:
        _NC_CACHE[key] = build_nc(S=S)
    nc = _NC_CACHE[key]

    in_maps = make_in_maps(q, k, v, mask, Wq, bq, Wk, bk, Wv, bv, Wo, bo)
    res = run_bass_kernel_spmd(
        nc, in_maps, core_ids=list(range(len(in_maps))), trace=TRACE)
    LAST_EXEC_NS = res.exec_time_ns
    LAST_RESULTS = res
    outs = [r["out"] for r in res.results]
    bo_np = np.asarray(bo, np.float32)
    full = np.zeros((B, S, D), np.float32)
    for b in range(B):
        acc = outs[b * 4].astype(np.float32).copy()
        for hg in range(1, 4):
            acc += outs[b * 4 + hg]
        full[b] = acc + bo_np
    return full


# revision 22
# speedup vs baseline: 1.7937x; 1.1443x over previous
"""Trainium2 Bass kernel for multi-head attention (B=2, S=2048, D=1024, H=16).

Sharding: 8 cores = 2 batches x 4 head-groups (Megatron column-parallel QKV /
row-parallel Wo). Each core computes a partial (S, D) output for its batch;
the host sums the 4 head-group partials per batch and adds bo.

Per-core pipeline (all on one NeuronCore):
  - QKV projections as fp32r matmuls (full rate, FP22 mantissa).
  - Scores computed transposed (t on partitions, s on free dim):
      scoresT[t, s] = sum_d khT[d, t] * qhT[d, s]
  - exp on ScalarE with the 1/sqrt(dh) scale folded in; no max-subtraction
    (scores ~ N(0,1) after scaling, exp can't overflow).
  - mask applied multiplicatively after exp (mask is 0/1) on VectorE in bf16.
  - PV matmul with a ones-augmented V (65th column) so the softmax
    denominator accumulates in the same PSUM tile.
  - normalize with DVE reciprocal + partition-broadcast DMA, then the Wo
    projection on-device.
"""

import os
import sys

for p in ("/opt/trn_rl_repo", "/root/.axon_site", "/root/.axon_site/_ro/trn_rl_repo"):
    if os.path.isdir(p) and p not in sys.path:
        sys.path.append(p)

import numpy as np
import ml_dtypes

import concourse.bass as bass
import concourse.mybir as mybir
import concourse.tile as tile
from concourse import bacc
from concourse.bass import ts, ds
from concourse.bass_utils import run_bass_kernel_spmd

F32 = mybir.dt.float32
F32R = mybir.dt.float32r
BF16 = mybir.dt.bfloat16

D = 1024          # model dim
DH = 64           # head dim
HPC = 4           # heads per core
DSH = HPC * DH    # sharded projection dim (256)
P = 128

AF = mybir.ActivationFunctionType


def build_nc(S=2048, n_cores=8):
    """Build the single-core SPMD Bass program."""
    SC = 512                  # s-chunk (matmul free dim)
    N_SC = S // SC            # s-chunks
    TT = S // P               # t tiles of 128
    G = 2                     # t-tiles per exp group (2 PSUM banks)
    N_G = TT // G
    KD = D // P               # contraction tiles for projections

    nc = bacc.Bacc(
        trn_type="TRN2",
        target_bir_lowering=False,
        debug=False,
        enable_asserts=False,
        num_devices=n_cores,
    )

    # DRAM I/O (per-core shard shapes)
    qT = nc.dram_tensor("qT", [D, S], BF16, kind="ExternalInput").ap()
    kT = nc.dram_tensor("kT", [D, S], BF16, kind="ExternalInput").ap()
    vT = nc.dram_tensor("vT", [D, S], BF16, kind="ExternalInput").ap()
    maskT = nc.dram_tensor("maskT", [S, S], BF16, kind="ExternalInput").ap()
    WqT = nc.dram_tensor("WqT", [D, DSH], BF16, kind="ExternalInput").ap()
    WkT = nc.dram_tensor("WkT", [D, DSH], BF16, kind="ExternalInput").ap()
    WvT = nc.dram_tensor("WvT", [D, DSH], BF16, kind="ExternalInput").ap()
    WoT = nc.dram_tensor("WoT", [DSH, D], BF16, kind="ExternalInput").ap()
    bq = nc.dram_tensor("bq", [DSH], F32, kind="ExternalInput").ap()
    bk = nc.dram_tensor("bk", [DSH], F32, kind="ExternalInput").ap()
    bv = nc.dram_tensor("bv", [DSH], F32, kind="ExternalInput").ap()
    out = nc.dram_tensor("out", [S, D], F32, kind="ExternalOutput").ap()

    with tile.TileContext(nc) as tc:
        _emit(tc, nc, S, SC, N_SC, TT, G, N_G, KD,
              qT, kT, vT, maskT, WqT, WkT, WvT, WoT, bq, bk, bv, out)
    nc.compile()
    return nc


def _emit(tc, nc, S, SC, N_SC, TT, G, N_G, KD,
          qT, kT, vT, maskT, WqT, WkT, WvT, WoT, bq, bk, bv, out):
    from contextlib import ExitStack

    from concourse import library_config

    ctx = ExitStack()
    with ctx:
        consts = ctx.enter_context(tc.tile_pool(name="consts", bufs=1))
        dram_scratch = ctx.enter_context(
            tc.tile_pool(name="dscratch", bufs=2, space="DRAM"))
        xstream = ctx.enter_context(tc.tile_pool(name="xstream", bufs=2))
        mask_pool = ctx.enter_context(tc.tile_pool(name="mask", bufs=2))
        attn_pool = ctx.enter_context(tc.tile_pool(name="attn", bufs=3))
        small = ctx.enter_context(tc.tile_pool(name="small", bufs=2))
        norm1 = ctx.enter_context(tc.tile_pool(name="norm1", bufs=1))
        out_pool = ctx.enter_context(tc.tile_pool(name="outp", bufs=2))

        # ---- resident tensors ----
        wq_sb = consts.tile([P, KD, DSH], BF16)
        nc.sync.dma_start(wq_sb, WqT.rearrange("(kt p) m -> p kt m", p=P))
        wk_sb = consts.tile([P, KD, DSH], BF16)
        nc.sync.dma_start(wk_sb, WkT.rearrange("(kt p) m -> p kt m", p=P))
        wv_sb = consts.tile([P, KD, DSH], BF16)
        nc.sync.dma_start(wv_sb, WvT.rearrange("(kt p) m -> p kt m", p=P))
        # WoT [DSH, D] -> [64, head, D]
        wo_sb = consts.tile([DH, HPC, D], BF16)
        nc.sync.dma_start(wo_sb, WoT.rearrange("(h p) n -> p h n", p=DH))

        bq_sb = consts.tile([P, DSH // P], F32)
        nc.sync.dma_start(bq_sb, bq.rearrange("(c p) -> p c", p=P))
        bk_sb = consts.tile([P, DSH // P], F32)
        nc.sync.dma_start(bk_sb, bk.rearrange("(c p) -> p c", p=P))
        # bv broadcast along partitions (added along the free dim of vh)
        bv_sb = consts.tile([P, DSH], F32)
        nc.sync.dma_start(bv_sb, bv[None, :].to_broadcast([P, DSH]))

        qhT = consts.tile([P, DSH // P, S], BF16)   # [dh%128, dh//128, s]
        khT = consts.tile([P, DSH // P, S], BF16)
        # V with ones column per head: [t-part, t-tile, h*65+d]
        vh_aug = consts.tile([P, TT, HPC * (DH + 1)], BF16)
        for h in range(HPC):
            nc.vector.memset(vh_aug[:, :, h * 65 + 64: h * 65 + 65], 1.0)
        # normalized attention outputs, heads on free dim: [64, h, s]
        ao_all = consts.tile([DH, HPC, S], BF16)
        ones64f = consts.tile([1, DH], F32)
        nc.vector.memset(ones64f, 1.0)
        ones64 = consts.tile([1, DH], F32R)
        nc.scalar.copy(ones64, ones64f)

        qT_t = qT.rearrange("(kt p) s -> p kt s", p=P)
        kT_t = kT.rearrange("(kt p) s -> p kt s", p=P)
        vT_t = vT.rearrange("(kt p) s -> p kt s", p=P)

        # ---- phase A: projections ----
        with tc.tile_pool(name="psA", bufs=3, space="PSUM") as psA:
            for (x_t, w_sb, b_sb, xhT) in ((qT_t, wq_sb, bq_sb, qhT),
                                           (kT_t, wk_sb, bk_sb, khT)):
                for sc in range(N_SC):
                    stage = xstream.tile([P, KD, SC], BF16, tag="xstage")
                    nc.sync.dma_start(stage, x_t[:, :, ts(sc, SC)])
                    pss = [psA.tile([P, SC], F32, tag="psA", name=f"psA{c}")
                           for c in range(2)]
                    for kt in range(KD):
                        for c in range(2):
                            nc.tensor.matmul(
                                pss[c], w_sb[:, kt, ts(c, P)], stage[:, kt, :],
                                start=(kt == 0), stop=(kt == KD - 1))
                    for c in range(2):
                        nc.scalar.activation(
                            xhT[:, c, ts(sc, SC)], pss[c], AF.Identity,
                            bias=b_sb[:, c:c + 1], scale=1.0)

            # V projection: vh[t, dh] orientation
            for tg in range(TT // 4):
                vstage = xstream.tile([P, KD, 4 * P], BF16, tag="vstage")
                nc.sync.dma_start(vstage, vT_t[:, :, ts(tg, 4 * P)])
                for tt4 in range(4):
                    tt = tg * 4 + tt4
                    psv = psA.tile([P, DSH], F32, tag="psA")
                    for kt in range(KD):
                        nc.tensor.matmul(psv, vstage[:, kt, ts(tt4, P)],
                                         wv_sb[:, kt, :],
                                         start=(kt == 0), stop=(kt == KD - 1))
                    for h in range(HPC):
                        nc.vector.tensor_add(
                            vh_aug[:, tt, h * 65: h * 65 + 64],
                            psv[:, ts(h, DH)], bv_sb[:, ts(h, DH)])

        # ---- phases B+C ----
        scores_pool = ctx.enter_context(
            tc.tile_pool(name="psScores", bufs=2, space="PSUM"))
        ao_pool = ctx.enter_context(
            tc.tile_pool(name="psAo", bufs=2, space="PSUM"))
        psC = ctx.enter_context(
            tc.tile_pool(name="psC", bufs=2, space="PSUM"))

        maskT_t = maskT.rearrange("(tt p) s -> p tt s", p=P)
        for sc in range(N_SC):
            mask_sb = mask_pool.tile([P, TT, SC], BF16)
            nc.sync.dma_start(mask_sb, maskT_t[:, :, ts(sc, SC)])

            ao_un = small.tile([DH, HPC, SC], F32, tag="ao_un")
            den4 = norm1.tile([1, HPC, SC], F32, tag="den4")

            def emit_qk(h, g):
                hp = (h % 2) * DH     # partition base within chunk
                c = h // 2
                sc_ps = scores_pool.tile([P, G, SC], F32, tag="scps", name=f"scps{h}_{g}")
                for i in range(G):
                    tt = g * G + i
                    nc.tensor.matmul(
                        sc_ps[:, i, :],
                        khT[hp:hp + DH, c, ts(tt, P)],
                        qhT[hp:hp + DH, c, ts(sc, SC)],
                        start=True, stop=True)
                return sc_ps

            for h in range(HPC):
                ao_ps = ao_pool.tile([DH + 1, SC], F32)
                # software-pipeline the QK stream one group ahead of PV so
                # the in-order PE queue never waits for exp/mask
                sc_ps = emit_qk(h, 0)
                for g in range(N_G):
                    attn = attn_pool.tile([P, G, SC], BF16)
                    nc.scalar.activation(attn, sc_ps, AF.Exp, scale=0.125)
                    nc.vector.tensor_mul(attn, attn,
                                         mask_sb[:, ts(g, G), :])
                    if g + 1 < N_G:
                        sc_ps = emit_qk(h, g + 1)
                    for i in range(G):
                        tt = g * G + i
                        nc.tensor.matmul(
                            ao_ps, vh_aug[:, tt, h * 65: h * 65 + 65],
                            attn[:, i, :],
                            start=(tt == 0), stop=(tt == TT - 1))
                # evacuate PSUM right away (frees the ao bank quickly)
                nc.vector.tensor_copy(ao_un[:, h, :], ao_ps[0:DH, :])
                nc.vector.tensor_copy(den4[:, h, :], ao_ps[DH:DH + 1, :])

            # batched normalization for all 4 heads: 1/x = exp(-ln(x)) on
            # ScalarE (one Ln->Exp table swap per s-chunk instead of per
            # head; DVE reciprocal would stall the mask-mult stream and the
            # custom-DVE approx is broken on HW). Broadcast across the 64
            # partitions with a tiny ones^T @ recip matmul (fp32r).
            lnd = norm1.tile([1, HPC, SC], F32, tag="lnd")
            nc.scalar.activation(lnd, den4, AF.Ln)
            recip = norm1.tile([1, HPC, SC], F32R, tag="recip")
            nc.scalar.activation(recip, lnd, AF.Exp, scale=-1.0)
            for h in range(HPC):
                rb = psC.tile([DH, SC], F32, tag="ops", name=f"rb{h}")
                nc.tensor.matmul(rb, ones64[:, :], recip[:, h, :],
                                 start=True, stop=True)
                nc.vector.tensor_mul(ao_all[:, h, ts(sc, SC)],
                                     ao_un[:, h, :], rb)

            # output projection for this s-chunk
            for st in range(SC // P):
                s0 = sc * SC + st * P
                for do in range(D // SC):
                    ops = psC.tile([P, SC], F32)
                    for h in range(HPC):
                        nc.tensor.matmul(
                            ops, ao_all[:, h, ds(s0, P)],
                            wo_sb[:, h, ts(do, SC)],
                            start=(h == 0), stop=(h == HPC - 1))
                    ot = out_pool.tile([P, SC], F32)
                    nc.vector.tensor_copy(ot, ops)
                    nc.sync.dma_start(out[ds(s0, P), ts(do, SC)], ot)


_NC_CACHE = {}

# test.py can set this to request a profiled run; exec time lands in
# LAST_EXEC_NS after the call.
TRACE = False
LAST_EXEC_NS = None
LAST_RESULTS = None


def make_in_maps(q, k, v, mask, Wq, bq, Wk, bk, Wv, bv, Wo, bo):
    B = q.shape[0]
    in_maps = []
    for b in range(B):
        qTb = np.ascontiguousarray(np.asarray(q[b], np.float32).T).astype(ml_dtypes.bfloat16)
        kTb = np.ascontiguousarray(np.asarray(k[b], np.float32).T).astype(ml_dtypes.bfloat16)
        vTb = np.ascontiguousarray(np.asarray(v[b], np.float32).T).astype(ml_dtypes.bfloat16)
        mTb = np.ascontiguousarray(
            np.asarray(mask[b, 0], np.float32).T).astype(ml_dtypes.bfloat16)
        for hg in range(4):
            sl = slice(hg * DSH, (hg + 1) * DSH)
            in_maps.append({
                "qT": qTb, "kT": kTb, "vT": vTb, "maskT": mTb,
                "WqT": np.ascontiguousarray(np.asarray(Wq, np.float32)[sl].T).astype(ml_dtypes.bfloat16),
                "WkT": np.ascontiguousarray(np.asarray(Wk, np.float32)[sl].T).astype(ml_dtypes.bfloat16),
                "WvT": np.ascontiguousarray(np.asarray(Wv, np.float32)[sl].T).astype(ml_dtypes.bfloat16),
                "WoT": np.ascontiguousarray(np.asarray(Wo, np.float32)[:, sl].T).astype(ml_dtypes.bfloat16),
                "bq": np.asarray(bq, np.float32)[sl].copy(),
                "bk": np.asarray(bk, np.float32)[sl].copy(),
                "bv": np.asarray(bv, np.float32)[sl].copy(),
            })
    return in_maps


def kernel(q, k, v, mask, Wq, bq, Wk, bk, Wv, bv, Wo, bo):
    global LAST_EXEC_NS, LAST_RESULTS
    q = np.asarray(q)
    B, S, _ = q.shape
    key = S
    if key not in _NC_CACHE:
        _NC_CACHE[key] = build_nc(S=S)
    nc = _NC_CACHE[key]

    in_maps = make_in_maps(q, k, v, mask, Wq, bq, Wk, bk, Wv, bv, Wo, bo)
    res = run_bass_kernel_spmd(
        nc, in_maps, core_ids=list(range(len(in_maps))), trace=TRACE)
    LAST_EXEC_NS = res.exec_time_ns
    LAST_RESULTS = res
    outs = [r["out"] for r in res.results]
    bo_np = np.asarray(bo, np.float32)
    full = np.zeros((B, S, D), np.float32)
    for b in range(B):
        acc = outs[b * 4].astype(np.float32).copy()
        for hg in range(1, 4):
            acc += outs[b * 4 + hg]
        full[b] = acc + bo_np
    return full
